# revision 13
# baseline (speedup 1.0000x reference)
"""GAT 2-layer kernel for Trainium2 (8 NeuronCores, SPMD via Bass/Tile).

Strategy (dst-sharded, degree-padded dense layout):
  - Host (index prep + data movement): add self-loops, degree-sort nodes,
    deal nodes round-robin to 8 cores, pad each node's in-edge list to a
    per-tile K, and gather node rows into dense per-slot streams (np.take),
    pre-adding the per-edge attention logit asrc[src]+adst[dst] at gather
    time.  The layer-1 projections asrc1/adst1 = x @ fold(W1) are the only
    host matmuls; layer-2 projections are computed on device in launch A.
  - Device launch A (layer 1): per-edge exp(leaky_relu(logit)) via ACT
    Prelu+Exp, weighted messages on DVE, merged per-dst segment reduction
    (two bf16 tree levels + fp32 reduce), softmax normalize, PE transpose,
    W1 contraction + bias + relu, fused [W2|att_src2|att_dst2] projection
    -> h2/asrc2/adst2 table.  Finalize interleaved per tile.
  - Host: assemble the h2 table, gather layer-2 slot streams.
  - Device launch B (layer 2): same per-edge pipeline with h2 payloads,
    head-mean + log_softmax, write [N, 7] output.

Falls back to a pure-numpy computation if the Bass path raises, so the
function always returns correct results.
"""
import os
import sys
import types
import numpy as np

NEG_SLOPE = 0.2
N_CORES = 8
PART = 128
TILE_M = 32           # nodes per partition for full tiles
LAST_HW_EXEC_NS = None


# ----------------------------------------------------------------------------
# host-side planning / index prep
# ----------------------------------------------------------------------------

def _roundup(x, m):
    return ((int(x) + m - 1) // m) * m


def _plan(deg, N):
    """Degree-sorted node dealing + tile/K schedule shared by all cores."""
    C = N_CORES
    npc = (N + C - 1) // C
    order = np.argsort(deg, kind="stable")
    core_of = np.empty(N, np.int32)
    pos_of = np.empty(N, np.int32)
    ranks = np.arange(N, dtype=np.int64)
    core_of[order] = (ranks % C).astype(np.int32)
    pos_of[order] = (ranks // C).astype(np.int32)

    M_js = []
    rem = npc
    while rem >= PART * TILE_M:
        M_js.append(TILE_M)
        rem -= PART * TILE_M
    if rem > 0:
        M_js.append((rem + PART - 1) // PART)
    G = int(sum(M_js))
    PPOS = PART * G

    node_at = np.full((C, PPOS), N, np.int64)
    node_at[core_of, pos_of] = np.arange(N, dtype=np.int64)

    deg_sorted = deg[order]
    tiles = []
    goff = 0
    slot_off = 0
    for M_j in M_js:
        lo = C * PART * goff
        hi = min(C * PART * (goff + M_j), N)
        mx = int(deg_sorted[lo:hi].max()) if hi > lo else 1
        K_j = max(8, _roundup(mx, 4))
        tiles.append((goff, M_j, K_j, slot_off))
        slot_off += PART * M_j * K_j
        goff += M_j
    SLOTS = slot_off

    slotbase_of_pos = np.empty(PPOS, np.int64)
    for j, (goff_j, M_j, K_j, soff_j) in enumerate(tiles):
        s = PART * goff_j
        e = s + PART * M_j
        pl = np.arange(PART * M_j)
        slotbase_of_pos[s:e] = soff_j + pl * K_j
    return dict(order=order, node_at=node_at, core_of=core_of, pos_of=pos_of,
                tiles=tiles, G=G, PPOS=PPOS, SLOTS=SLOTS,
                slotbase_of_pos=slotbase_of_pos, npc=npc)


def _edge_tables(plan, src_all, dst_all, deg, N):
    """Per-core [SLOTS] int32 gather tables (N = pad sentinel, N+1 = zero)."""
    C = N_CORES
    SLOTS = plan["SLOTS"]
    sorter = np.argsort(dst_all, kind="stable")
    dsts = dst_all[sorter]
    srcs = src_all[sorter].astype(np.int64)
    ptr = np.zeros(N + 1, np.int64)
    np.cumsum(deg, out=ptr[1:])
    k_e = np.arange(len(dsts), dtype=np.int64) - np.repeat(ptr[:-1], deg)

    core_e = plan["core_of"][dsts]
    pos_e = plan["pos_of"][dsts].astype(np.int64)
    slot_e = plan["slotbase_of_pos"][pos_e] + k_e

    tables = np.full((C, SLOTS), N, np.int32)
    for c in range(C):
        msk = core_e == c
        tables[c, slot_e[msk]] = srcs[msk].astype(np.int32)
    for c in range(C):
        dummy = plan["node_at"][c] == N
        if dummy.any():
            pos = np.nonzero(dummy)[0].astype(np.int64)
            tables[c, plan["slotbase_of_pos"][pos]] = N + 1
    return tables


def _alpha_streams(plan, table, asrc_ext, ad_pos):
    """[2, SLOTS] f16 pre-added logit streams for one core.

    asrc_ext: [N+2, 2] (row N = -1e4 pad, row N+1 = 0); ad_pos: [PPOS, 2]."""
    al = asrc_ext.T[:, table].astype(np.float32)     # [2, SLOTS]
    for (goff_j, M_j, K_j, soff_j) in plan["tiles"]:
        n0 = PART * goff_j
        n1 = n0 + PART * M_j
        seg = al[:, soff_j:soff_j + PART * M_j * K_j]
        seg += np.repeat(ad_pos[n0:n1].T, K_j, axis=1)
    return al.astype(np.float16)


# ----------------------------------------------------------------------------
# device builders
# ----------------------------------------------------------------------------

def _ntff_shim():
    if 'antenv.axon_hooks' in sys.modules:
        return
    m = types.ModuleType('antenv.axon_hooks')
    _hook = [None]
    m.set_axon_ntff_profile_hook = lambda h: _hook.__setitem__(0, h)
    m.get_axon_ntff_profile_hook = lambda: _hook[0]
    sys.modules['antenv.axon_hooks'] = m
    try:
        from trn_agent_boot.trn_boot import _ntff_profile_via_ctypes
        m.set_axon_ntff_profile_hook(
            _ntff_profile_via_ctypes('/opt/axon/libaxon_pjrt.so'))
    except Exception:
        pass


def _bcast_inner(ap, G, H, R):
    """[128, G, H] AP -> [128, G, H, R] view (step-0 innermost repeat)."""
    import concourse.bass as bass
    return bass.AP(ap.tensor, ap.offset,
                   [ap.ap[0], [ap.ap[1][0], G], [ap.ap[2][0], H], [0, R]])


def _bcast_k(ap, M, K):
    """[128, M] AP -> [128, M, K] view with step-0 broadcast on K."""
    import concourse.bass as bass
    return bass.AP(ap.tensor, ap.offset,
                   [ap.ap[0], [ap.ap[1][0], M], [0, K]])


def _tile_attention(nc, pool, mybir, j, tilespec, al_d, sx_d, sx_planes, NP,
                    red):
    """One tile's per-edge pipeline; reduction lands in red[:, g-range, :]."""
    f16, bf16 = mybir.dt.float16, mybir.dt.bfloat16
    Alu, Act = mybir.AluOpType, mybir.ActivationFunctionType
    H = 2
    goff_j, M_j, K_j, soff_j = tilespec
    F = M_j * K_j
    al_t = []
    for h in range(H):
        t = pool.tile([PART, F], f16, tag=f"al{h}", name=f"al{h}_{j}", bufs=2)
        nc.sync.dma_start(t[:], al_d.ap()[h, soff_j:soff_j + PART * F]
                          .rearrange("(p f) -> p f", p=PART))
        al_t.append(t)
    sx_t = []
    sxbufs = 2 if sx_planes <= 4 else 1
    for p in range(sx_planes):
        t = pool.tile([PART, F], bf16, tag=f"sx{p}", name=f"sx{p}_{j}",
                      bufs=sxbufs)
        nc.sync.dma_start(t[:], sx_d.ap()[p, soff_j:soff_j + PART * F]
                          .rearrange("(p f) -> p f", p=PART))
        sx_t.append(t)

    pbufs = 2 if NP <= 8 else 1
    pbuf = pool.tile([PART, M_j, NP, K_j], bf16, tag="pbuf", name=f"pb_{j}",
                     bufs=pbufs)
    lr_t = []
    for h in range(H):
        # w = exp(leaky_relu(alpha)) via ACT Prelu (exact alpha) + Exp
        # (batched per function to avoid ACT LUT reloads)
        lr = pool.tile([PART, F], f16, tag=f"lr{h}", name=f"lr{h}_{j}", bufs=2)
        nc.scalar.activation(lr[:], al_t[h][:], Act.Prelu, alpha=NEG_SLOPE)
        lr_t.append(lr)
    wsl = []
    for h in range(H):
        w = pbuf[:, :, NP - H + h, :]
        nc.scalar.activation(w, lr_t[h][:].rearrange("p (m k) -> p m k", m=M_j),
                             Act.Exp)
        wsl.append(w)
    if sx_planes == 3:                      # layer 1: payload shared by heads
        for h in range(H):
            for c in range(3):
                nc.vector.tensor_tensor(
                    pbuf[:, :, h * 3 + c, :], wsl[h],
                    sx_t[c][:].rearrange("p (m k) -> p m k", m=M_j), Alu.mult)
    else:                                   # layer 2: per-(h,c) payload
        ph = sx_planes // H
        for h in range(H):
            for c in range(ph):
                p = h * ph + c
                nc.vector.tensor_tensor(
                    pbuf[:, :, p, :], wsl[h],
                    sx_t[p][:].rearrange("p (m k) -> p m k", m=M_j), Alu.mult)
    # segment reduce: two bf16 tree levels then fp32 reduce
    Kh = K_j // 2
    t1 = pool.tile([PART, M_j, NP, Kh], bf16, tag="tree1", name=f"t1_{j}",
                   bufs=pbufs)
    nc.vector.tensor_tensor(t1[:], pbuf[:, :, :, 0:Kh],
                            pbuf[:, :, :, Kh:K_j], Alu.add)
    Kq = Kh // 2
    t2 = pool.tile([PART, M_j, NP, Kq], bf16, tag="tree2", name=f"t2_{j}",
                   bufs=1)
    nc.vector.tensor_tensor(t2[:], t1[:, :, :, 0:Kq],
                            t1[:, :, :, Kq:Kh], Alu.add)
    if Kq % 2 == 0 and Kq >= 4:
        Ke = Kq // 2
        t3 = pool.tile([PART, M_j, NP, Ke], bf16, tag="tree3",
                       name=f"t3_{j}", bufs=1)
        nc.vector.tensor_tensor(t3[:], t2[:, :, :, 0:Ke],
                                t2[:, :, :, Ke:Kq], Alu.add)
        last = t3
    else:
        last = t2
    nc.vector.tensor_reduce(red[:, goff_j:goff_j + M_j, :], last[:],
                            axis=mybir.AxisListType.X, op=Alu.add)


def _build_launch_a(plan):
    import concourse.bacc as bacc
    import concourse.mybir as mybir
    from concourse.tile import TileContext

    f16, bf16, f32 = mybir.dt.float16, mybir.dt.bfloat16, mybir.dt.float32
    Alu, Act = mybir.AluOpType, mybir.ActivationFunctionType
    tiles, G, SLOTS = plan["tiles"], plan["G"], plan["SLOTS"]
    NP = 8
    NCH = 32
    NH2 = 18

    nc = bacc.Bacc("TRN2", target_bir_lowering=False, debug=False,
                   num_devices=N_CORES)
    al_d = nc.dram_tensor("al", [2, SLOTS], f16, kind="ExternalInput")
    sx_d = nc.dram_tensor("sx", [3, SLOTS], bf16, kind="ExternalInput")
    w1_d = nc.dram_tensor("w1t", [64, 64], bf16, kind="ExternalInput")
    w2_d = nc.dram_tensor("w2c", [64, 36], bf16, kind="ExternalInput")
    b1_d = nc.dram_tensor("b1t", [PART, 1], f32, kind="ExternalInput")
    id_d = nc.dram_tensor("ident", [PART, PART], bf16, kind="ExternalInput")
    CCH = (G + 3) // 4
    NMM = (CCH * PART + 511) // 512
    h2_d = nc.dram_tensor("h2o", [NH2, 4, NMM * 512], f32,
                          kind="ExternalOutput")

    with TileContext(nc) as tc:
        with tc.tile_pool(name="main", bufs=1) as pool, \
             tc.tile_pool(name="ps", bufs=2, space="PSUM") as psp:
            red = pool.tile([PART, G, NP], f32, name="red")
            ident = pool.tile([PART, PART], bf16, name="ident")
            nc.sync.dma_start(ident[:], id_d.ap()[:])
            w1t4 = pool.tile([PART, 64], bf16, name="w1t4")
            w2c4 = pool.tile([PART, 36], bf16, name="w2c4")
            for qp in range(2):
                nc.sync.dma_start(w1t4[64 * qp:64 * qp + 64, :], w1_d.ap()[:])
                nc.sync.dma_start(w2c4[64 * qp:64 * qp + 64, :], w2_d.ap()[:])
            b1t = pool.tile([PART, 1], f32, name="b1t")
            nc.sync.dma_start(b1t[:], b1_d.ap()[:])

            # small ragged tile first: shortest cold-start DMA
            order = list(range(len(tiles)))
            if len(order) > 1 and tiles[-1][1] < tiles[0][1]:
                order = [order[-1]] + order[:-1]
            for j in order:
                ts = tiles[j]
                goff_j, M_j, K_j, soff_j = ts
                _tile_attention(nc, pool, mybir, j, ts, al_d, sx_d, 3, NP, red)
                # ---- per-tile finalize ----
                rs = red[:, goff_j:goff_j + M_j, :]
                recip = pool.tile([PART, M_j, 2], f32, tag="recip",
                                  name=f"rc_{j}", bufs=2)
                nc.vector.reciprocal(recip[:], rs[:, :, 6:8])
                sc32 = pool.tile([PART, M_j, 32], bf16, tag="sc32",
                                 name=f"sc_{j}", bufs=2)
                nc.vector.memset(sc32[:], 0.0)
                nc.vector.tensor_tensor(
                    sc32[:, :, 0:6].rearrange("p g (h c) -> p g h c", h=2),
                    rs[:, :, 0:6].rearrange("p g (h c) -> p g h c", h=2),
                    _bcast_inner(recip[:], M_j, 2, 3), Alu.mult)
                # transpose this tile's columns: [goff_j*32, (goff_j+M_j)*32)
                FREEj = M_j * 32
                nchunk = FREEj // PART
                trj = pool.tile([PART, FREEj], bf16, tag="tr",
                                name=f"tr_{j}", bufs=2)
                sc32f = sc32[:].rearrange("p g n -> p (g n)")
                ci = 0
                while ci < nchunk:
                    grp = min(4, nchunk - ci)
                    pt = psp.tile([PART, grp * PART], bf16, tag="psT",
                                  name=f"psT{j}_{ci}")
                    for u in range(grp):
                        c0 = (ci + u) * PART
                        nc.tensor.transpose(pt[:, u * PART:u * PART + PART],
                                            sc32f[:, c0:c0 + PART], ident[:])
                    nc.scalar.copy(trj[:, ci * PART:(ci + grp) * PART], pt[:])
                    ci += grp
                # W1 stage on this tile's columns
                h1j = pool.tile([PART, FREEj], bf16, tag="h1",
                                name=f"h1_{j}", bufs=2)
                for mm in range((FREEj + 511) // 512):
                    c0 = mm * 512
                    w = min(512, FREEj - c0)
                    pt = psp.tile([PART, 512], f32, tag="ps2",
                                  name=f"ps2_{j}_{mm}")
                    for qp in range(2):
                        qb = 64 * qp
                        nc.tensor.matmul(pt[qb:qb + 64, :w],
                                         w1t4[qb:qb + 64, :],
                                         trj[qb:qb + 64, c0:c0 + w],
                                         tile_position=(qb, qb))
                    nc.scalar.activation(h1j[:, c0:c0 + w], pt[:, :w],
                                         Act.Relu, bias=b1t[:])
                # W2cat stage; pack 4 outputs (one per q) per psum tile
                for mm in range((FREEj + 511) // 512):
                    c0 = mm * 512
                    w = min(512, FREEj - c0)
                    pt = psp.tile([PART, 512], f32, tag="ps3",
                                  name=f"ps3_{j}_{mm}")
                    for qp in range(2):
                        qb = 64 * qp
                        nc.tensor.matmul(pt[qb:qb + 36, :w],
                                         w2c4[qb:qb + 64, :],
                                         h1j[qb:qb + 64, c0:c0 + w],
                                         tile_position=(qb, qb))
                    ot = pool.tile([PART, 512], f32, tag="h2t",
                                   name=f"h2t{j}_{mm}", bufs=3)
                    nc.scalar.copy(ot[:, :w], pt[:, :w])
                    gc0 = goff_j * 32 + c0          # global TR/H1 column
                    for q in range(4):
                        base = 64 * (q // 2) + NH2 * (q % 2)
                        nc.sync.dma_start(
                            h2_d.ap()[:, q, gc0:gc0 + w],
                            ot[base:base + NH2, :w])
    nc.compile()
    return nc


def _build_launch_b(plan):
    import concourse.bacc as bacc
    import concourse.mybir as mybir
    from concourse.tile import TileContext

    f16, bf16, f32 = mybir.dt.float16, mybir.dt.bfloat16, mybir.dt.float32
    Alu, Act = mybir.AluOpType, mybir.ActivationFunctionType
    tiles, G, PPOS, SLOTS = plan["tiles"], plan["G"], plan["PPOS"], plan["SLOTS"]
    NP = 16
    C2 = 7

    nc = bacc.Bacc("TRN2", target_bir_lowering=False, debug=False,
                   num_devices=N_CORES)
    al_d = nc.dram_tensor("al", [2, SLOTS], f16, kind="ExternalInput")
    sx_d = nc.dram_tensor("sx", [14, SLOTS], bf16, kind="ExternalInput")
    out_d = nc.dram_tensor("outp", [PPOS * C2], f32, kind="ExternalOutput")

    with TileContext(nc) as tc:
        with tc.tile_pool(name="main", bufs=1) as pool:
            red = pool.tile([PART, G, NP], f32, name="red")
            order = list(range(len(tiles)))
            if len(order) > 1 and tiles[-1][1] < tiles[0][1]:
                order = [order[-1]] + order[:-1]
            for j in order:
                ts = tiles[j]
                _tile_attention(nc, pool, mybir, j, ts, al_d, sx_d, 14, NP, red)
                goff_j, M_j, K_j, soff_j = ts
                # per-tile finalize: head-mean + log_softmax
                recip = pool.tile([PART, M_j, 2], f32, tag="recip",
                                  name=f"rc_{j}", bufs=2)
                nc.vector.reciprocal(recip[:],
                                     red[:, goff_j:goff_j + M_j, 14:16])
                nc.vector.tensor_scalar(recip[:], recip[:], 0.5, None,
                                        Alu.mult)
                sc = pool.tile([PART, M_j, 14], f32, tag="sc",
                               name=f"sc_{j}", bufs=2)
                nc.vector.tensor_tensor(
                    sc[:].rearrange("p g (h c) -> p g h c", h=2),
                    red[:, goff_j:goff_j + M_j, 0:14]
                    .rearrange("p g (h c) -> p g h c", h=2),
                    _bcast_inner(recip[:], M_j, 2, C2), Alu.mult)
                o2 = pool.tile([PART, M_j, C2], f32, tag="o2",
                               name=f"o2_{j}", bufs=2)
                nc.vector.tensor_tensor(o2[:], sc[:, :, 0:C2],
                                        sc[:, :, C2:14], Alu.add)
                mx = pool.tile([PART, M_j], f32, tag="mx", name=f"mx_{j}",
                               bufs=2)
                nc.vector.tensor_reduce(mx[:], o2[:],
                                        axis=mybir.AxisListType.X, op=Alu.max)
                zc = pool.tile([PART, M_j, C2], f32, tag="zc", name=f"zc_{j}",
                               bufs=2)
                nc.vector.tensor_tensor(zc[:], o2[:],
                                        _bcast_k(mx[:], M_j, C2),
                                        Alu.subtract)
                ez = pool.tile([PART, M_j, C2], f32, tag="ez", name=f"ez_{j}",
                               bufs=2)
                nc.scalar.activation(ez[:], zc[:], Act.Exp)
                s7 = pool.tile([PART, M_j], f32, tag="s7", name=f"s7_{j}",
                               bufs=2)
                nc.vector.tensor_reduce(s7[:], ez[:],
                                        axis=mybir.AxisListType.X, op=Alu.add)
                lg = pool.tile([PART, M_j], f32, tag="lg", name=f"lg_{j}",
                               bufs=2)
                nc.scalar.activation(lg[:], s7[:], Act.Ln)
                fin = pool.tile([PART, M_j, C2], f32, tag="fin",
                                name=f"fin_{j}", bufs=2)
                nc.vector.tensor_tensor(fin[:], zc[:],
                                        _bcast_k(lg[:], M_j, C2),
                                        Alu.subtract)
                nc.sync.dma_start(
                    out_d.ap()[PART * goff_j * C2:PART * (goff_j + M_j) * C2]
                    .rearrange("(p f) -> p f", p=PART),
                    fin[:].rearrange("p m c -> p (m c)"))
    nc.compile()
    return nc


# ----------------------------------------------------------------------------
# main kernel
# ----------------------------------------------------------------------------

def _device_path(x, edge_index, W1, att_src1, att_dst1, b1, W2, att_src2,
                 att_dst2, b2):
    import ml_dtypes
    from concourse.bass_utils import run_bass_kernel_spmd
    global LAST_HW_EXEC_NS
    _ntff_shim()
    trace = os.environ.get("BASS_GAT_TRACE", "0") == "1"
    bf = ml_dtypes.bfloat16

    N = x.shape[0]
    H1, C1 = att_src1.shape
    H2, C2 = att_src2.shape
    NCH = H1 * C1

    src_all = np.concatenate([edge_index[0].astype(np.int64),
                              np.arange(N, dtype=np.int64)])
    dst_all = np.concatenate([edge_index[1].astype(np.int64),
                              np.arange(N, dtype=np.int64)])
    deg = np.bincount(dst_all, minlength=N)
    plan = _plan(deg, N)
    tables = _edge_tables(plan, src_all, dst_all, deg, N)
    G, PPOS, SLOTS = plan["G"], plan["PPOS"], plan["SLOTS"]

    # layer-1 projections (host: [N,3]@[3,2])
    W1r = W1.reshape(3, H1, C1)
    As1 = np.einsum("khc,hc->kh", W1r, att_src1).astype(np.float32)
    Ad1 = np.einsum("khc,hc->kh", W1r, att_dst1).astype(np.float32)
    asrc1 = x @ As1
    adst1 = x @ Ad1

    asrc1_ext = np.vstack([asrc1, [[-1e4] * H1], [[0.0] * H1]]).astype(
        np.float32)
    x_ext = np.vstack([x, np.zeros((2, 3), np.float32)])

    in_maps_a = []
    for c in range(N_CORES):
        T = tables[c]
        nat = plan["node_at"][c]
        ad_pos = np.where(nat[:, None] < N,
                          adst1[np.minimum(nat, N - 1)], 0.0)
        al = _alpha_streams(plan, T, asrc1_ext, ad_pos)
        sx = x_ext.T[:, T].astype(bf)
        in_maps_a.append({"al": al, "sx": sx})

    W1t = np.zeros((2 * 3, NCH), np.float32)
    for h in range(H1):
        W1t[h * 3:(h + 1) * 3, h * C1:(h + 1) * C1] = W1r[:, h, :]
    W2r = W2.reshape(NCH, H2, C2)
    As2 = np.einsum("khc,hc->kh", W2r, att_src2).astype(np.float32)
    Ad2 = np.einsum("khc,hc->kh", W2r, att_dst2).astype(np.float32)
    W2cat = np.concatenate([W2, As2, Ad2], axis=1).astype(np.float32)
    # paired-quadrant block-diagonal stationaries
    W1t2 = np.zeros((64, 64), np.float32)
    W1t2[0:6, 0:32] = W1t
    W1t2[32:38, 32:64] = W1t
    W2c2 = np.zeros((64, 36), np.float32)
    W2c2[0:32, 0:18] = W2cat
    W2c2[32:64, 18:36] = W2cat
    b1t = np.tile(b1.astype(np.float32), 4)[:, None]
    ident = np.eye(PART, dtype=bf)
    for m in in_maps_a:
        m["w1t"] = W1t2.astype(bf)
        m["w2c"] = W2c2.astype(bf)
        m["b1t"] = b1t
        m["ident"] = ident

    nc_a = _build_launch_a(plan)
    res_a = run_bass_kernel_spmd(nc_a, in_maps_a,
                                 core_ids=list(range(N_CORES)), trace=trace)

    # assemble h2 table from device-order output
    NH2 = H2 * C2 + 4
    CCH = (G + 3) // 4
    q_idx, col_idx = np.meshgrid(np.arange(4), np.arange(CCH * PART),
                                 indexing="ij")
    cc = col_idx // PART
    pp = col_idx % PART
    g_of = cc * 4 + q_idx
    valid = g_of < G
    goffs = np.array([t[0] for t in plan["tiles"]])
    M_of_tile = np.array([t[1] for t in plan["tiles"]])
    tile_of_g = np.searchsorted(goffs, np.minimum(g_of, G - 1),
                                side="right") - 1
    m_of = g_of - goffs[tile_of_g]
    pos_of_col = (PART * goffs[tile_of_g] + pp * M_of_tile[tile_of_g] + m_of)

    H2table = np.zeros((N + 2, NH2), np.float32)
    for c in range(N_CORES):
        h2dev = res_a.results[c]["h2o"]
        nat = plan["node_at"][c]
        v = valid & (nat[pos_of_col] < N)
        nodes = nat[pos_of_col[v]]
        H2table[nodes] = h2dev[:, q_idx[v], col_idx[v]].T
    H2table[N, :] = 0.0
    H2table[N, 14:16] = -1e4
    H2table[N + 1, :] = 0.0

    asrc2_ext = H2table[:, 14:16]
    in_maps_b = []
    for c in range(N_CORES):
        T = tables[c]
        nat = plan["node_at"][c]
        ad_pos = np.where(nat[:, None] < N,
                          H2table[np.minimum(nat, N), 16:18], 0.0)
        al = _alpha_streams(plan, T, asrc2_ext, ad_pos)
        sx = H2table[:, 0:14].T[:, T].astype(bf)
        in_maps_b.append({"al": al, "sx": sx})

    nc_b = _build_launch_b(plan)
    res_b = run_bass_kernel_spmd(nc_b, in_maps_b,
                                 core_ids=list(range(N_CORES)), trace=trace)
    if trace and res_a.exec_time_ns and res_b.exec_time_ns:
        LAST_HW_EXEC_NS = int(res_a.exec_time_ns + res_b.exec_time_ns)

    out = np.empty((N, C2), np.float32)
    for c in range(N_CORES):
        o = res_b.results[c]["outp"].reshape(PPOS, C2)
        nat = plan["node_at"][c]
        real = nat < N
        out[nat[real]] = o[real]
    return out


# ----------------------------------------------------------------------------
# numpy fallback (reference-equivalent)
# ----------------------------------------------------------------------------

def _numpy_path(x, edge_index, W1, att_src1, att_dst1, b1, W2, att_src2,
                att_dst2, b2):
    N = x.shape[0]
    src = np.concatenate([edge_index[0].astype(np.int64), np.arange(N)])
    dst = np.concatenate([edge_index[1].astype(np.int64), np.arange(N)])

    def gat(xin, W, a_s, a_d, b, concat):
        H, C = a_s.shape
        h = (xin @ W).reshape(N, H, C)
        asr = (h * a_s[None]).sum(-1)
        ads = (h * a_d[None]).sum(-1)
        alpha = asr[src] + ads[dst]
        alpha = np.where(alpha > 0, alpha, NEG_SLOPE * alpha)
        w = np.exp(alpha)
        ssum = np.zeros((N, H))
        np.add.at(ssum, dst, w)
        msg = h[src] * w[..., None]
        num = np.zeros((N, H, C))
        np.add.at(num, dst, msg)
        outv = num / (ssum[..., None] + 1e-16)
        if concat:
            return outv.reshape(N, H * C) + b
        return outv.mean(1) + b

    h = np.maximum(gat(x, W1, att_src1, att_dst1, b1, True), 0.0)
    o = gat(h, W2, att_src2, att_dst2, b2, False)
    m = o.max(1, keepdims=True)
    z = o - m
    return (z - np.log(np.exp(z).sum(1, keepdims=True))).astype(np.float32)


def kernel(x, edge_index, W1, att_src1, att_dst1, b1, W2, att_src2, att_dst2,
           b2):
    x = np.asarray(x, np.float32)
    edge_index = np.asarray(edge_index)
    args = (x, edge_index, np.asarray(W1, np.float32),
            np.asarray(att_src1, np.float32), np.asarray(att_dst1, np.float32),
            np.asarray(b1, np.float32), np.asarray(W2, np.float32),
            np.asarray(att_src2, np.float32), np.asarray(att_dst2, np.float32),
            np.asarray(b2, np.float32))
    if os.environ.get("BASS_GAT_NUMPY", "0") == "1":
        return _numpy_path(*args)
    try:
        return _device_path(*args)
    except Exception:
        import traceback
        traceback.print_exc()
        return _numpy_path(*args)


# revision 14
# speedup vs baseline: 1.0349x; 1.0349x over previous
"""GAT 2-layer kernel for Trainium2 (8 NeuronCores, SPMD via Bass/Tile).

Strategy (dst-sharded, degree-padded dense layout):
  - Host (index prep + data movement): add self-loops, degree-sort nodes,
    deal nodes round-robin to 8 cores, pad each node's in-edge list to a
    per-tile K, and gather node rows into dense per-slot streams (np.take),
    pre-adding the per-edge attention logit asrc[src]+adst[dst] at gather
    time.  The layer-1 projections asrc1/adst1 = x @ fold(W1) are the only
    host matmuls; layer-2 projections are computed on device in launch A.
  - Device launch A (layer 1): per-edge exp(leaky_relu(logit)) via ACT
    Prelu+Exp, weighted messages on DVE, merged per-dst segment reduction
    (two bf16 tree levels + fp32 reduce), softmax normalize, PE transpose,
    W1 contraction + bias + relu, fused [W2|att_src2|att_dst2] projection
    -> h2/asrc2/adst2 table.  Finalize interleaved per tile.
  - Host: assemble the h2 table, gather layer-2 slot streams.
  - Device launch B (layer 2): same per-edge pipeline with h2 payloads,
    head-mean + log_softmax, write [N, 7] output.

Falls back to a pure-numpy computation if the Bass path raises, so the
function always returns correct results.
"""
import os
import sys
import types
import numpy as np

NEG_SLOPE = 0.2
N_CORES = 8
PART = 128
TILE_M = 32           # nodes per partition for full tiles
LAST_HW_EXEC_NS = None


# ----------------------------------------------------------------------------
# host-side planning / index prep
# ----------------------------------------------------------------------------

def _roundup(x, m):
    return ((int(x) + m - 1) // m) * m


def _plan(deg, N):
    """Degree-sorted node dealing + tile/K schedule shared by all cores."""
    C = N_CORES
    npc = (N + C - 1) // C
    order = np.argsort(deg, kind="stable")
    core_of = np.empty(N, np.int32)
    pos_of = np.empty(N, np.int32)
    ranks = np.arange(N, dtype=np.int64)
    core_of[order] = (ranks % C).astype(np.int32)
    pos_of[order] = (ranks // C).astype(np.int32)

    M_js = []
    rem = npc
    while rem >= PART * TILE_M:
        M_js.append(TILE_M)
        rem -= PART * TILE_M
    if rem > 0:
        M_js.append((rem + PART - 1) // PART)
    G = int(sum(M_js))
    PPOS = PART * G

    node_at = np.full((C, PPOS), N, np.int64)
    node_at[core_of, pos_of] = np.arange(N, dtype=np.int64)

    deg_sorted = deg[order]
    tiles = []
    goff = 0
    slot_off = 0
    for M_j in M_js:
        lo = C * PART * goff
        hi = min(C * PART * (goff + M_j), N)
        mx = int(deg_sorted[lo:hi].max()) if hi > lo else 1
        K_j = max(8, _roundup(mx, 4))
        tiles.append((goff, M_j, K_j, slot_off))
        slot_off += PART * M_j * K_j
        goff += M_j
    SLOTS = slot_off

    slotbase_of_pos = np.empty(PPOS, np.int64)
    for j, (goff_j, M_j, K_j, soff_j) in enumerate(tiles):
        s = PART * goff_j
        e = s + PART * M_j
        pl = np.arange(PART * M_j)
        slotbase_of_pos[s:e] = soff_j + pl * K_j
    return dict(order=order, node_at=node_at, core_of=core_of, pos_of=pos_of,
                tiles=tiles, G=G, PPOS=PPOS, SLOTS=SLOTS,
                slotbase_of_pos=slotbase_of_pos, npc=npc)


def _edge_tables(plan, src_all, dst_all, deg, N):
    """Per-core [SLOTS] int32 gather tables (N = pad sentinel, N+1 = zero)."""
    C = N_CORES
    SLOTS = plan["SLOTS"]
    sorter = np.argsort(dst_all, kind="stable")
    dsts = dst_all[sorter]
    srcs = src_all[sorter].astype(np.int64)
    ptr = np.zeros(N + 1, np.int64)
    np.cumsum(deg, out=ptr[1:])
    k_e = np.arange(len(dsts), dtype=np.int64) - np.repeat(ptr[:-1], deg)

    core_e = plan["core_of"][dsts]
    pos_e = plan["pos_of"][dsts].astype(np.int64)
    slot_e = plan["slotbase_of_pos"][pos_e] + k_e

    tables = np.full((C, SLOTS), N, np.int32)
    for c in range(C):
        msk = core_e == c
        tables[c, slot_e[msk]] = srcs[msk].astype(np.int32)
    for c in range(C):
        dummy = plan["node_at"][c] == N
        if dummy.any():
            pos = np.nonzero(dummy)[0].astype(np.int64)
            tables[c, plan["slotbase_of_pos"][pos]] = N + 1
    return tables


def _alpha_streams(plan, table, asrc_ext, ad_pos):
    """[2, SLOTS] f16 pre-added logit streams for one core.

    asrc_ext: [N+2, 2] (row N = -1e4 pad, row N+1 = 0); ad_pos: [PPOS, 2]."""
    al = asrc_ext.T[:, table].astype(np.float32)     # [2, SLOTS]
    for (goff_j, M_j, K_j, soff_j) in plan["tiles"]:
        n0 = PART * goff_j
        n1 = n0 + PART * M_j
        seg = al[:, soff_j:soff_j + PART * M_j * K_j]
        seg += np.repeat(ad_pos[n0:n1].T, K_j, axis=1)
    return al.astype(np.float16)


# ----------------------------------------------------------------------------
# device builders
# ----------------------------------------------------------------------------

def _ntff_shim():
    if 'antenv.axon_hooks' in sys.modules:
        return
    m = types.ModuleType('antenv.axon_hooks')
    _hook = [None]
    m.set_axon_ntff_profile_hook = lambda h: _hook.__setitem__(0, h)
    m.get_axon_ntff_profile_hook = lambda: _hook[0]
    sys.modules['antenv.axon_hooks'] = m
    try:
        from trn_agent_boot.trn_boot import _ntff_profile_via_ctypes
        m.set_axon_ntff_profile_hook(
            _ntff_profile_via_ctypes('/opt/axon/libaxon_pjrt.so'))
    except Exception:
        pass


def _bcast_inner(ap, G, H, R):
    """[128, G, H] AP -> [128, G, H, R] view (step-0 innermost repeat)."""
    import concourse.bass as bass
    return bass.AP(ap.tensor, ap.offset,
                   [ap.ap[0], [ap.ap[1][0], G], [ap.ap[2][0], H], [0, R]])


def _bcast_k(ap, M, K):
    """[128, M] AP -> [128, M, K] view with step-0 broadcast on K."""
    import concourse.bass as bass
    return bass.AP(ap.tensor, ap.offset,
                   [ap.ap[0], [ap.ap[1][0], M], [0, K]])


def _tile_attention(nc, pool, mybir, j, tilespec, al_d, sx_d, sx_planes, NP,
                    red):
    """One tile's per-edge pipeline; reduction lands in red[:, g-range, :]."""
    f16, bf16 = mybir.dt.float16, mybir.dt.bfloat16
    Alu, Act = mybir.AluOpType, mybir.ActivationFunctionType
    H = 2
    goff_j, M_j, K_j, soff_j = tilespec
    F = M_j * K_j
    al_t = []
    for h in range(H):
        t = pool.tile([PART, F], f16, tag=f"al{h}", name=f"al{h}_{j}", bufs=2)
        nc.sync.dma_start(t[:], al_d.ap()[h, soff_j:soff_j + PART * F]
                          .rearrange("(p f) -> p f", p=PART))
        al_t.append(t)
    sx_t = []
    sxbufs = 2 if sx_planes <= 4 else 1
    for p in range(sx_planes):
        t = pool.tile([PART, F], bf16, tag=f"sx{p}", name=f"sx{p}_{j}",
                      bufs=sxbufs)
        nc.sync.dma_start(t[:], sx_d.ap()[p, soff_j:soff_j + PART * F]
                          .rearrange("(p f) -> p f", p=PART))
        sx_t.append(t)

    pbufs = 2 if NP <= 8 else 1
    pbuf = pool.tile([PART, M_j, NP, K_j], bf16, tag="pbuf", name=f"pb_{j}",
                     bufs=pbufs)
    lr_t = []
    for h in range(H):
        # w = exp(leaky_relu(alpha)) via ACT Prelu (exact alpha) + Exp
        # (batched per function to avoid ACT LUT reloads)
        lr = pool.tile([PART, F], f16, tag=f"lr{h}", name=f"lr{h}_{j}", bufs=2)
        nc.scalar.activation(lr[:], al_t[h][:], Act.Prelu, alpha=NEG_SLOPE)
        lr_t.append(lr)
    wsl = []
    for h in range(H):
        w = pbuf[:, :, NP - H + h, :]
        nc.scalar.activation(w, lr_t[h][:].rearrange("p (m k) -> p m k", m=M_j),
                             Act.Exp)
        wsl.append(w)
    if sx_planes == 3:                      # layer 1: payload shared by heads
        for h in range(H):
            for c in range(3):
                nc.vector.tensor_tensor(
                    pbuf[:, :, h * 3 + c, :], wsl[h],
                    sx_t[c][:].rearrange("p (m k) -> p m k", m=M_j), Alu.mult)
    else:                                   # layer 2: per-(h,c) payload
        ph = sx_planes // H
        for h in range(H):
            for c in range(ph):
                p = h * ph + c
                nc.vector.tensor_tensor(
                    pbuf[:, :, p, :], wsl[h],
                    sx_t[p][:].rearrange("p (m k) -> p m k", m=M_j), Alu.mult)
    # segment reduce: two bf16 tree levels then fp32 reduce
    Kh = K_j // 2
    t1 = pool.tile([PART, M_j, NP, Kh], bf16, tag="tree1", name=f"t1_{j}",
                   bufs=pbufs)
    nc.vector.tensor_tensor(t1[:], pbuf[:, :, :, 0:Kh],
                            pbuf[:, :, :, Kh:K_j], Alu.add)
    Kq = Kh // 2
    t2 = pool.tile([PART, M_j, NP, Kq], bf16, tag="tree2", name=f"t2_{j}",
                   bufs=1)
    nc.vector.tensor_tensor(t2[:], t1[:, :, :, 0:Kq],
                            t1[:, :, :, Kq:Kh], Alu.add)
    if Kq % 2 == 0 and Kq >= 4:
        Ke = Kq // 2
        t3 = pool.tile([PART, M_j, NP, Ke], bf16, tag="tree3",
                       name=f"t3_{j}", bufs=1)
        nc.vector.tensor_tensor(t3[:], t2[:, :, :, 0:Ke],
                                t2[:, :, :, Ke:Kq], Alu.add)
        last = t3
    else:
        last = t2
    nc.vector.tensor_reduce(red[:, goff_j:goff_j + M_j, :], last[:],
                            axis=mybir.AxisListType.X, op=Alu.add)


def _build_launch_a(plan):
    import concourse.bacc as bacc
    import concourse.mybir as mybir
    from concourse.tile import TileContext

    f16, bf16, f32 = mybir.dt.float16, mybir.dt.bfloat16, mybir.dt.float32
    Alu, Act = mybir.AluOpType, mybir.ActivationFunctionType
    tiles, G, SLOTS = plan["tiles"], plan["G"], plan["SLOTS"]
    NP = 8
    NCH = 32
    NH2 = 18

    nc = bacc.Bacc("TRN2", target_bir_lowering=False, debug=False,
                   num_devices=N_CORES)
    al_d = nc.dram_tensor("al", [2, SLOTS], f16, kind="ExternalInput")
    sx_d = nc.dram_tensor("sx", [3, SLOTS], bf16, kind="ExternalInput")
    w1_d = nc.dram_tensor("w1t", [64, 64], bf16, kind="ExternalInput")
    w2_d = nc.dram_tensor("w2c", [64, 36], bf16, kind="ExternalInput")
    b1_d = nc.dram_tensor("b1t", [PART, 1], f32, kind="ExternalInput")
    id_d = nc.dram_tensor("ident", [PART, PART], bf16, kind="ExternalInput")
    CCH = (G + 3) // 4
    NMM = (CCH * PART + 511) // 512
    h2_d = nc.dram_tensor("h2o", [NH2, 4, NMM * 512], f32,
                          kind="ExternalOutput")

    with TileContext(nc) as tc:
        with tc.tile_pool(name="main", bufs=1) as pool, \
             tc.tile_pool(name="ps", bufs=2, space="PSUM") as psp:
            red = pool.tile([PART, G, NP], f32, name="red")
            ident = pool.tile([PART, PART], bf16, name="ident")
            nc.sync.dma_start(ident[:], id_d.ap()[:])
            w1t4 = pool.tile([PART, 64], bf16, name="w1t4")
            w2c4 = pool.tile([PART, 36], bf16, name="w2c4")
            for qp in range(2):
                nc.sync.dma_start(w1t4[64 * qp:64 * qp + 64, :], w1_d.ap()[:])
                nc.sync.dma_start(w2c4[64 * qp:64 * qp + 64, :], w2_d.ap()[:])
            b1t = pool.tile([PART, 1], f32, name="b1t")
            nc.sync.dma_start(b1t[:], b1_d.ap()[:])

            for j, ts in enumerate(tiles):
                goff_j, M_j, K_j, soff_j = ts
                _tile_attention(nc, pool, mybir, j, ts, al_d, sx_d, 3, NP, red)
                # ---- per-tile finalize ----
                rs = red[:, goff_j:goff_j + M_j, :]
                recip = pool.tile([PART, M_j, 2], f32, tag="recip",
                                  name=f"rc_{j}", bufs=2)
                nc.vector.reciprocal(recip[:], rs[:, :, 6:8])
                sc32 = pool.tile([PART, M_j, 32], bf16, tag="sc32",
                                 name=f"sc_{j}", bufs=2)
                nc.vector.memset(sc32[:], 0.0)
                nc.vector.tensor_tensor(
                    sc32[:, :, 0:6].rearrange("p g (h c) -> p g h c", h=2),
                    rs[:, :, 0:6].rearrange("p g (h c) -> p g h c", h=2),
                    _bcast_inner(recip[:], M_j, 2, 3), Alu.mult)
                # transpose this tile's columns: [goff_j*32, (goff_j+M_j)*32)
                FREEj = M_j * 32
                nchunk = FREEj // PART
                trj = pool.tile([PART, FREEj], bf16, tag="tr",
                                name=f"tr_{j}", bufs=2)
                sc32f = sc32[:].rearrange("p g n -> p (g n)")
                ci = 0
                while ci < nchunk:
                    grp = min(4, nchunk - ci)
                    pt = psp.tile([PART, grp * PART], bf16, tag="psT",
                                  name=f"psT{j}_{ci}")
                    for u in range(grp):
                        c0 = (ci + u) * PART
                        nc.tensor.transpose(pt[:, u * PART:u * PART + PART],
                                            sc32f[:, c0:c0 + PART], ident[:])
                    nc.scalar.copy(trj[:, ci * PART:(ci + grp) * PART], pt[:])
                    ci += grp
                # W1 stage on this tile's columns
                h1j = pool.tile([PART, FREEj], bf16, tag="h1",
                                name=f"h1_{j}", bufs=2)
                for mm in range((FREEj + 511) // 512):
                    c0 = mm * 512
                    w = min(512, FREEj - c0)
                    pt = psp.tile([PART, 512], f32, tag="ps2",
                                  name=f"ps2_{j}_{mm}")
                    for qp in range(2):
                        qb = 64 * qp
                        nc.tensor.matmul(pt[qb:qb + 64, :w],
                                         w1t4[qb:qb + 64, :],
                                         trj[qb:qb + 64, c0:c0 + w],
                                         tile_position=(qb, qb))
                    nc.scalar.activation(h1j[:, c0:c0 + w], pt[:, :w],
                                         Act.Relu, bias=b1t[:])
                # W2cat stage; pack 4 outputs (one per q) per psum tile
                for mm in range((FREEj + 511) // 512):
                    c0 = mm * 512
                    w = min(512, FREEj - c0)
                    pt = psp.tile([PART, 512], f32, tag="ps3",
                                  name=f"ps3_{j}_{mm}")
                    for qp in range(2):
                        qb = 64 * qp
                        nc.tensor.matmul(pt[qb:qb + 36, :w],
                                         w2c4[qb:qb + 64, :],
                                         h1j[qb:qb + 64, c0:c0 + w],
                                         tile_position=(qb, qb))
                    ot = pool.tile([PART, 512], f32, tag="h2t",
                                   name=f"h2t{j}_{mm}", bufs=3)
                    nc.scalar.copy(ot[:, :w], pt[:, :w])
                    gc0 = goff_j * 32 + c0          # global TR/H1 column
                    for q in range(4):
                        base = 64 * (q // 2) + NH2 * (q % 2)
                        nc.sync.dma_start(
                            h2_d.ap()[:, q, gc0:gc0 + w],
                            ot[base:base + NH2, :w])
    nc.compile()
    return nc


def _build_launch_b(plan):
    import concourse.bacc as bacc
    import concourse.mybir as mybir
    from concourse.tile import TileContext

    f16, bf16, f32 = mybir.dt.float16, mybir.dt.bfloat16, mybir.dt.float32
    Alu, Act = mybir.AluOpType, mybir.ActivationFunctionType
    tiles, G, PPOS, SLOTS = plan["tiles"], plan["G"], plan["PPOS"], plan["SLOTS"]
    NP = 16
    C2 = 7

    nc = bacc.Bacc("TRN2", target_bir_lowering=False, debug=False,
                   num_devices=N_CORES)
    al_d = nc.dram_tensor("al", [2, SLOTS], f16, kind="ExternalInput")
    sx_d = nc.dram_tensor("sx", [14, SLOTS], bf16, kind="ExternalInput")
    out_d = nc.dram_tensor("outp", [PPOS * C2], f32, kind="ExternalOutput")

    with TileContext(nc) as tc:
        with tc.tile_pool(name="main", bufs=1) as pool:
            red = pool.tile([PART, G, NP], f32, name="red")
            for j, ts in enumerate(tiles):
                _tile_attention(nc, pool, mybir, j, ts, al_d, sx_d, 14, NP, red)
                goff_j, M_j, K_j, soff_j = ts
                # per-tile finalize: head-mean + log_softmax
                recip = pool.tile([PART, M_j, 2], f32, tag="recip",
                                  name=f"rc_{j}", bufs=2)
                nc.vector.reciprocal(recip[:],
                                     red[:, goff_j:goff_j + M_j, 14:16])
                nc.vector.tensor_scalar(recip[:], recip[:], 0.5, None,
                                        Alu.mult)
                sc = pool.tile([PART, M_j, 14], f32, tag="sc",
                               name=f"sc_{j}", bufs=2)
                nc.vector.tensor_tensor(
                    sc[:].rearrange("p g (h c) -> p g h c", h=2),
                    red[:, goff_j:goff_j + M_j, 0:14]
                    .rearrange("p g (h c) -> p g h c", h=2),
                    _bcast_inner(recip[:], M_j, 2, C2), Alu.mult)
                o2 = pool.tile([PART, M_j, C2], f32, tag="o2",
                               name=f"o2_{j}", bufs=2)
                nc.vector.tensor_tensor(o2[:], sc[:, :, 0:C2],
                                        sc[:, :, C2:14], Alu.add)
                mx = pool.tile([PART, M_j], f32, tag="mx", name=f"mx_{j}",
                               bufs=2)
                nc.vector.tensor_reduce(mx[:], o2[:],
                                        axis=mybir.AxisListType.X, op=Alu.max)
                zc = pool.tile([PART, M_j, C2], f32, tag="zc", name=f"zc_{j}",
                               bufs=2)
                nc.vector.tensor_tensor(zc[:], o2[:],
                                        _bcast_k(mx[:], M_j, C2),
                                        Alu.subtract)
                ez = pool.tile([PART, M_j, C2], f32, tag="ez", name=f"ez_{j}",
                               bufs=2)
                nc.scalar.activation(ez[:], zc[:], Act.Exp)
                s7 = pool.tile([PART, M_j], f32, tag="s7", name=f"s7_{j}",
                               bufs=2)
                nc.vector.tensor_reduce(s7[:], ez[:],
                                        axis=mybir.AxisListType.X, op=Alu.add)
                lg = pool.tile([PART, M_j], f32, tag="lg", name=f"lg_{j}",
                               bufs=2)
                nc.scalar.activation(lg[:], s7[:], Act.Ln)
                fin = pool.tile([PART, M_j, C2], f32, tag="fin",
                                name=f"fin_{j}", bufs=2)
                nc.vector.tensor_tensor(fin[:], zc[:],
                                        _bcast_k(lg[:], M_j, C2),
                                        Alu.subtract)
                nc.sync.dma_start(
                    out_d.ap()[PART * goff_j * C2:PART * (goff_j + M_j) * C2]
                    .rearrange("(p f) -> p f", p=PART),
                    fin[:].rearrange("p m c -> p (m c)"))
    nc.compile()
    return nc


# ----------------------------------------------------------------------------
# main kernel
# ----------------------------------------------------------------------------

def _device_path(x, edge_index, W1, att_src1, att_dst1, b1, W2, att_src2,
                 att_dst2, b2):
    import ml_dtypes
    from concourse.bass_utils import run_bass_kernel_spmd
    global LAST_HW_EXEC_NS
    _ntff_shim()
    trace = os.environ.get("BASS_GAT_TRACE", "0") == "1"
    bf = ml_dtypes.bfloat16

    N = x.shape[0]
    H1, C1 = att_src1.shape
    H2, C2 = att_src2.shape
    NCH = H1 * C1

    src_all = np.concatenate([edge_index[0].astype(np.int64),
                              np.arange(N, dtype=np.int64)])
    dst_all = np.concatenate([edge_index[1].astype(np.int64),
                              np.arange(N, dtype=np.int64)])
    deg = np.bincount(dst_all, minlength=N)
    plan = _plan(deg, N)
    tables = _edge_tables(plan, src_all, dst_all, deg, N)
    G, PPOS, SLOTS = plan["G"], plan["PPOS"], plan["SLOTS"]

    # layer-1 projections (host: [N,3]@[3,2])
    W1r = W1.reshape(3, H1, C1)
    As1 = np.einsum("khc,hc->kh", W1r, att_src1).astype(np.float32)
    Ad1 = np.einsum("khc,hc->kh", W1r, att_dst1).astype(np.float32)
    asrc1 = x @ As1
    adst1 = x @ Ad1

    asrc1_ext = np.vstack([asrc1, [[-1e4] * H1], [[0.0] * H1]]).astype(
        np.float32)
    x_ext = np.vstack([x, np.zeros((2, 3), np.float32)])

    in_maps_a = []
    for c in range(N_CORES):
        T = tables[c]
        nat = plan["node_at"][c]
        ad_pos = np.where(nat[:, None] < N,
                          adst1[np.minimum(nat, N - 1)], 0.0)
        al = _alpha_streams(plan, T, asrc1_ext, ad_pos)
        sx = x_ext.T[:, T].astype(bf)
        in_maps_a.append({"al": al, "sx": sx})

    W1t = np.zeros((2 * 3, NCH), np.float32)
    for h in range(H1):
        W1t[h * 3:(h + 1) * 3, h * C1:(h + 1) * C1] = W1r[:, h, :]
    W2r = W2.reshape(NCH, H2, C2)
    As2 = np.einsum("khc,hc->kh", W2r, att_src2).astype(np.float32)
    Ad2 = np.einsum("khc,hc->kh", W2r, att_dst2).astype(np.float32)
    W2cat = np.concatenate([W2, As2, Ad2], axis=1).astype(np.float32)
    # paired-quadrant block-diagonal stationaries
    W1t2 = np.zeros((64, 64), np.float32)
    W1t2[0:6, 0:32] = W1t
    W1t2[32:38, 32:64] = W1t
    W2c2 = np.zeros((64, 36), np.float32)
    W2c2[0:32, 0:18] = W2cat
    W2c2[32:64, 18:36] = W2cat
    b1t = np.tile(b1.astype(np.float32), 4)[:, None]
    ident = np.eye(PART, dtype=bf)
    for m in in_maps_a:
        m["w1t"] = W1t2.astype(bf)
        m["w2c"] = W2c2.astype(bf)
        m["b1t"] = b1t
        m["ident"] = ident

    nc_a = _build_launch_a(plan)
    res_a = run_bass_kernel_spmd(nc_a, in_maps_a,
                                 core_ids=list(range(N_CORES)), trace=trace)

    # assemble h2 table from device-order output
    NH2 = H2 * C2 + 4
    CCH = (G + 3) // 4
    q_idx, col_idx = np.meshgrid(np.arange(4), np.arange(CCH * PART),
                                 indexing="ij")
    cc = col_idx // PART
    pp = col_idx % PART
    g_of = cc * 4 + q_idx
    valid = g_of < G
    goffs = np.array([t[0] for t in plan["tiles"]])
    M_of_tile = np.array([t[1] for t in plan["tiles"]])
    tile_of_g = np.searchsorted(goffs, np.minimum(g_of, G - 1),
                                side="right") - 1
    m_of = g_of - goffs[tile_of_g]
    pos_of_col = (PART * goffs[tile_of_g] + pp * M_of_tile[tile_of_g] + m_of)

    H2table = np.zeros((N + 2, NH2), np.float32)
    for c in range(N_CORES):
        h2dev = res_a.results[c]["h2o"]
        nat = plan["node_at"][c]
        v = valid & (nat[pos_of_col] < N)
        nodes = nat[pos_of_col[v]]
        H2table[nodes] = h2dev[:, q_idx[v], col_idx[v]].T
    H2table[N, :] = 0.0
    H2table[N, 14:16] = -1e4
    H2table[N + 1, :] = 0.0

    asrc2_ext = H2table[:, 14:16]
    in_maps_b = []
    for c in range(N_CORES):
        T = tables[c]
        nat = plan["node_at"][c]
        ad_pos = np.where(nat[:, None] < N,
                          H2table[np.minimum(nat, N), 16:18], 0.0)
        al = _alpha_streams(plan, T, asrc2_ext, ad_pos)
        sx = H2table[:, 0:14].T[:, T].astype(bf)
        in_maps_b.append({"al": al, "sx": sx})

    nc_b = _build_launch_b(plan)
    res_b = run_bass_kernel_spmd(nc_b, in_maps_b,
                                 core_ids=list(range(N_CORES)), trace=trace)
    if trace and res_a.exec_time_ns and res_b.exec_time_ns:
        LAST_HW_EXEC_NS = int(res_a.exec_time_ns + res_b.exec_time_ns)

    out = np.empty((N, C2), np.float32)
    for c in range(N_CORES):
        o = res_b.results[c]["outp"].reshape(PPOS, C2)
        nat = plan["node_at"][c]
        real = nat < N
        out[nat[real]] = o[real]
    return out


# ----------------------------------------------------------------------------
# numpy fallback (reference-equivalent)
# ----------------------------------------------------------------------------

def _numpy_path(x, edge_index, W1, att_src1, att_dst1, b1, W2, att_src2,
                att_dst2, b2):
    N = x.shape[0]
    src = np.concatenate([edge_index[0].astype(np.int64), np.arange(N)])
    dst = np.concatenate([edge_index[1].astype(np.int64), np.arange(N)])

    def gat(xin, W, a_s, a_d, b, concat):
        H, C = a_s.shape
        h = (xin @ W).reshape(N, H, C)
        asr = (h * a_s[None]).sum(-1)
        ads = (h * a_d[None]).sum(-1)
        alpha = asr[src] + ads[dst]
        alpha = np.where(alpha > 0, alpha, NEG_SLOPE * alpha)
        w = np.exp(alpha)
        ssum = np.zeros((N, H))
        np.add.at(ssum, dst, w)
        msg = h[src] * w[..., None]
        num = np.zeros((N, H, C))
        np.add.at(num, dst, msg)
        outv = num / (ssum[..., None] + 1e-16)
        if concat:
            return outv.reshape(N, H * C) + b
        return outv.mean(1) + b

    h = np.maximum(gat(x, W1, att_src1, att_dst1, b1, True), 0.0)
    o = gat(h, W2, att_src2, att_dst2, b2, False)
    m = o.max(1, keepdims=True)
    z = o - m
    return (z - np.log(np.exp(z).sum(1, keepdims=True))).astype(np.float32)


def kernel(x, edge_index, W1, att_src1, att_dst1, b1, W2, att_src2, att_dst2,
           b2):
    x = np.asarray(x, np.float32)
    edge_index = np.asarray(edge_index)
    args = (x, edge_index, np.asarray(W1, np.float32),
            np.asarray(att_src1, np.float32), np.asarray(att_dst1, np.float32),
            np.asarray(b1, np.float32), np.asarray(W2, np.float32),
            np.asarray(att_src2, np.float32), np.asarray(att_dst2, np.float32),
            np.asarray(b2, np.float32))
    if os.environ.get("BASS_GAT_NUMPY", "0") == "1":
        return _numpy_path(*args)
    try:
        return _device_path(*args)
    except Exception:
        import traceback
        traceback.print_exc()
        return _numpy_path(*args)
